# revision 45
# baseline (speedup 1.0000x reference)
"""Trainium2 Bass kernel for CrossAttention (sparse_attention variant).

Reference computation (shapes hardcoded):
  x [2, 1024, 1024], context [2, 4, 1024, 1024], doc_similarities [2, 4]
  q = x @ Wq, kv = ctx @ Wkv (k|v), dots = q k^T / sqrt(d) + doc_bias,
  attn = softmax(dots over all 4096 doc tokens), out = (attn @ v) @ Wout + bout

Sharding: 8 cores = 2 batches x 4 head-pairs.  Core c: batch c//4, heads
{2*(c%4), 2*(c%4)+1}.  Each core computes a [1024, 1024] partial of the
output projection (its heads' rows of Wout); host sums 4 partials per batch.

v4 design (per-core, activations bf16, QK in fp8e4 DoubleRow):
  Projections in bf16.  Inputs host-packed [128, KT, cols]; startup DMAs
  are ~256KB chunks issued in PE-consumption order so Kproj/Qproj start
  ~3us in.  q^T/k^T built packed [64|64 partitions(head), 2(hd-half),
  cols]; QK runs fp8 DoubleRow producing S^T [j, i] per head.  exp on
  ScalarE (fused scale + per-partition doc bias) -> bf16, paced into the
  projection phase.  V produced NATURALLY [j, 2*hd].  Softmax denominator:
  DVE fp16 sub-chain adds; per-head finalize = DVE merge-add + GPSIMD
  partition_all_reduce (dn broadcast to all partitions) + DVE reciprocal
  -- no PE/Act involvement.  Normalization via DVE multiply right after
  each head's last EV; out-projection per (i-tile, oc-half) with
  Act/DVE-split eviction and per-half writeback DMAs to shorten the tail.
"""

import numpy as np
import ml_dtypes
from contextlib import ExitStack

import concourse.bass as bass
import concourse.mybir as mybir
import concourse.tile as tile
from concourse import bacc
from concourse import bass_utils
from concourse.bass import bass_isa

# Problem constants
B, N, M, CN, D = 2, 1024, 4, 1024, 1024
H = 8          # total heads
HPC = 2        # heads per core
NCORES = 8
HD = D // H    # 128
J = M * CN     # 4096
KT = D // 128  # 8 contraction k-tiles
IC = N // 512  # 2 i-chunks of queries
JC = J // 512  # 8 j-chunks (projection granularity)
JT = J // 128  # 32 j-tiles (attention granularity)
NSUB = 1       # dn accumulator chains per head (DVE keeps up with 1)
SCALE = float(D ** -0.5)

FR = mybir.dt.float32r
F32 = mybir.dt.float32
BF16 = mybir.dt.bfloat16
FP16 = mybir.dt.float16
FP8 = mybir.dt.float8e4

PACE_C = 7     # QK+exp tiles drained per jc during the projection phase

_NC_CACHE = {}
LAST_RESULT = None


def _build_module():
    nc = bacc.Bacc(
        "TRN2",
        target_bir_lowering=False,
        debug=False,
        num_devices=NCORES,
    )

    xP = nc.dram_tensor("xP", [128, KT, N], FP16, kind="ExternalInput").ap()
    ctxP = nc.dram_tensor("ctxP", [128, KT, J], FP16, kind="ExternalInput").ap()
    wq = nc.dram_tensor("wq", [128, KT * HPC * HD], FP16, kind="ExternalInput").ap()
    wk = nc.dram_tensor("wk", [128, KT * HPC * HD], FP16, kind="ExternalInput").ap()
    wv = nc.dram_tensor("wv", [128, KT * HPC * HD], FP16, kind="ExternalInput").ap()
    wout = nc.dram_tensor("wout", [128, HPC * D], FP16, kind="ExternalInput").ap()
    docb = nc.dram_tensor("docb", [128, JT], F32, kind="ExternalInput").ap()
    outp = nc.dram_tensor("outp", [N, D], FP16, kind="ExternalOutput").ap()

    EXP = mybir.ActivationFunctionType.Exp
    COPY = mybir.ActivationFunctionType.Copy
    DR = mybir.MatmulPerfMode.DoubleRow

    with tile.TileContext(nc) as tc:
      with ExitStack() as ctx:
        wpool = ctx.enter_context(tc.tile_pool(name="wpool", bufs=1))
        big = ctx.enter_context(tc.tile_pool(name="big", bufs=1))
        ctp = ctx.enter_context(tc.tile_pool(name="ctp", bufs=1))
        etp = ctx.enter_context(tc.tile_pool(name="etp", bufs=1))
        ostream = ctx.enter_context(tc.tile_pool(name="ostream", bufs=5))
        stp = ctx.enter_context(tc.tile_pool(name="stp", bufs=2, space="PSUM"))

        # ---- weights / inputs ----
        # wq/wk: [128, KT, 2 halves, 128]: half-major column order per kt
        # (h0-lo, h1-lo | h0-hi, h1-hi) so each lo/hi slice is contiguous
        wq_sb = wpool.tile([128, KT, 2, HPC * 64], FP16, name="wq_sb")
        wk_sb = wpool.tile([128, KT, 2, HPC * 64], FP16, name="wk_sb")
        wv_sb = wpool.tile([128, KT, HPC * HD], FP16, name="wv_sb")
        wout_sb = wpool.tile([128, HPC, D], FP16, name="wout_sb")
        docb_sb = wpool.tile([128, JT], F32, name="docb_sb")
        xt_sb = wpool.tile([128, KT, N], FP16, name="xt_sb")

        # ---- persistent activations ----
        qT8 = big.tile([128, 2, N], FP8, name="qT8")      # [64|64 p, half, i]
        kT8 = big.tile([128, 2, J], FP8, name="kT8")      # [64|64 p, half, j]
        vnat = big.tile([128, JT, HPC * HD], FP16, name="vnat")  # v natural
        acc = big.tile([128, HPC, N], FP16, name="acc")   # dn partials
        yn = big.tile([128, HPC, N], FP16, name="yn")     # normalized y^T
        rs_sb = big.tile([128, HPC, N], F32, name="rs_sb")

        # ---- constants: tiny ones tile memset on the (idle) Pool engine so
        # a junk matmul can start the PE p-state ramp clock at ~0.7us ----
        ones_b = wpool.tile([128, 128], BF16, name="ones_b")
        nc.gpsimd.memset(ones_b[:, :], 1.0)

        # ---- DMA choreography: chunks in PE-consumption order ----
        # PE order: K0 V0 K1 V1 Q0 Q1, so x/Wq stay off the critical path.
        ct0 = ctp.tile([128, KT, 512], FP16, name="ct", tag="ct", bufs=3)
        nc.sync.dma_start(out=wk_sb[:, 0:2, :, :], in_=wk[:, 0:512])
        nc.sync.dma_start(out=ct0[:, 0:2, :], in_=ctxP[:, 0:2, 0:512])
        nc.sync.dma_start(out=wk_sb[:, 2:8, :, :], in_=wk[:, 512:2048])
        nc.sync.dma_start(out=ct0[:, 2:4, :], in_=ctxP[:, 2:4, 0:512])
        nc.sync.dma_start(out=ct0[:, 4:6, :], in_=ctxP[:, 4:6, 0:512])
        nc.sync.dma_start(out=ct0[:, 6:8, :], in_=ctxP[:, 6:8, 0:512])
        nc.sync.dma_start(out=wv_sb[:, 0:4, :], in_=wv[:, 0:1024])
        nc.sync.dma_start(out=wv_sb[:, 4:8, :], in_=wv[:, 1024:2048])
        ct_tiles = {0: ct0}

        def issue_ctx(jc, split=1):
            ct = ctp.tile([128, KT, 512], FP16, name="ct", tag="ct", bufs=3)
            step = KT // split
            for s in range(split):
                nc.sync.dma_start(
                    out=ct[:, s * step:(s + 1) * step, :],
                    in_=ctxP[:, s * step:(s + 1) * step,
                             jc * 512:(jc + 1) * 512])
            ct_tiles[jc] = ct

        issue_ctx(1, split=4)
        nc.sync.dma_start(out=wq_sb[:, :, :, :], in_=wq[:, :])
        nc.sync.dma_start(out=xt_sb[:, 0:4, 0:512], in_=xP[:, 0:4, 0:512])
        nc.sync.dma_start(out=xt_sb[:, 4:8, 0:512], in_=xP[:, 4:8, 0:512])
        nc.sync.dma_start(out=xt_sb[:, 0:4, 512:1024], in_=xP[:, 0:4, 512:1024])
        nc.sync.dma_start(out=xt_sb[:, 4:8, 512:1024], in_=xP[:, 4:8, 512:1024])
        nc.sync.dma_start(out=docb_sb[:, :], in_=docb[:, :])
        issue_ctx(2)
        issue_ctx(3)
        nc.sync.dma_start(out=wout_sb[:, :, :], in_=wout[:, :])

        pend = []
        et_tiles = {}
        addc = [0, 0]          # dn adds done per head
        head_fin = [False, False]

        with tc.tile_pool(name="proj", bufs=2, space="PSUM") as proj:

            def emit_qk_exp(h, jt):
                p0 = 64 * h
                st = stp.tile([128, N], F32, name="st", tag="st")
                for ic in range(IC):
                    nc.tensor.matmul(
                        st[:, ic * 512:(ic + 1) * 512],
                        lhsT=kT8[p0:p0 + 64, :, jt * 128:(jt + 1) * 128],
                        rhs=qT8[p0:p0 + 64, :, ic * 512:(ic + 1) * 512],
                        start=True, stop=True, perf_mode=DR)
                et = etp.tile([128, N], FP16, name="et", tag="et", bufs=44)
                nc.scalar.activation(et[:, :], st[:, :], EXP,
                                     bias=docb_sb[:, jt:jt + 1], scale=SCALE)
                et_tiles[(h, jt)] = et
                # dn chain add on DVE (fp16 acc, 2x mode)
                a = acc[:, h, :]
                if jt == 0:
                    nc.vector.tensor_copy(a, et[:, :])
                else:
                    nc.vector.tensor_add(a, a, et[:, :])
                addc[h] += 1

            def emit_kproj(jc):
                ct = ct_tiles[jc]
                kp0 = proj.tile([128, 512], F32, name="kp0", tag="kp")
                kp1 = proj.tile([128, 512], F32, name="kp1", tag="kp")
                for kt in range(KT):
                    nc.tensor.matmul(kp0[:, :], lhsT=wk_sb[:, kt, 0, :],
                                     rhs=ct[:, kt, :], start=(kt == 0),
                                     stop=(kt == KT - 1))
                    nc.tensor.matmul(kp1[:, :], lhsT=wk_sb[:, kt, 1, :],
                                     rhs=ct[:, kt, :], start=(kt == 0),
                                     stop=(kt == KT - 1))
                nc.vector.tensor_copy(kT8[:, 0, jc * 512:(jc + 1) * 512],
                                      kp0[:, :])
                nc.vector.tensor_copy(kT8[:, 1, jc * 512:(jc + 1) * 512],
                                      kp1[:, :])

            def emit_vproj(jc):
                ct = ct_tiles.pop(jc)
                for t in range(4):
                    jt = 4 * jc + t
                    vp = proj.tile([128, 512], F32, name="vp", tag="vp")
                    for kt in range(KT):
                        nc.tensor.matmul(
                            vp[:, 0:256],
                            lhsT=ct[:, kt, t * 128:(t + 1) * 128],
                            rhs=wv_sb[:, kt, :], start=(kt == 0),
                            stop=(kt == KT - 1))
                    nc.vector.tensor_copy(vnat[:, jt, :], vp[:, 0:256])

            def emit_qproj(ic, tag="vp"):
                qp0 = proj.tile([128, 512], F32, name="qp0", tag=tag)
                qp1 = proj.tile([128, 512], F32, name="qp1", tag=tag)
                for kt in range(KT):
                    nc.tensor.matmul(qp0[:, :], lhsT=wq_sb[:, kt, 0, :],
                                     rhs=xt_sb[:, kt, ic * 512:(ic + 1) * 512],
                                     start=(kt == 0), stop=(kt == KT - 1))
                    nc.tensor.matmul(qp1[:, :], lhsT=wq_sb[:, kt, 1, :],
                                     rhs=xt_sb[:, kt, ic * 512:(ic + 1) * 512],
                                     start=(kt == 0), stop=(kt == KT - 1))
                nc.vector.tensor_copy(qT8[:, 0, ic * 512:(ic + 1) * 512],
                                      qp0[:, :])
                nc.vector.tensor_copy(qT8[:, 1, ic * 512:(ic + 1) * 512],
                                      qp1[:, :])

            # PE ramp-clock starter: one junk matmul as early as possible
            # (the cost ramp is time-based from first PE activity, so this
            # makes every real matmul after ~3.7us run at the warm rate)
            warm = stp.tile([128, N], F32, name="st", tag="st")
            nc.tensor.matmul(warm[:, 0:128], lhsT=ones_b[:, :],
                             rhs=ones_b[:, :], start=True, stop=True)

            def pace(n):
                for _ in range(n):
                    if pend:
                        emit_qk_exp(*pend.pop(0))

            # jc0/jc1: K first (its DMAs land first), V next, Q last --
            # x/Wq arrive while the K/V work runs.
            emit_kproj(0)
            emit_vproj(0)
            emit_kproj(1)
            emit_vproj(1)
            emit_qproj(0)
            # Q1 rides the kp ring (K1's tiles are long evicted) so it does
            # not wait on Q0's own evictions through the vp ring.
            emit_qproj(1, tag="kp")
            for jc in range(2):
                for t in range(4):
                    pend.append((0, 4 * jc + t))
                for t in range(4):
                    pend.append((1, 4 * jc + t))

            for jc in range(2, JC):
                if jc + 2 < JC:
                    issue_ctx(jc + 2)
                # kproj with a pace point at the halfway mark: QK tiles are
                # spaced >= 850ns apart everywhere so the st ring (2 bufs)
                # never waits on the Act exp of the tile before last
                ct = ct_tiles[jc]
                kp0 = proj.tile([128, 512], F32, name="kp0", tag="kp")
                kp1 = proj.tile([128, 512], F32, name="kp1", tag="kp")
                for kt in range(KT):
                    nc.tensor.matmul(kp0[:, :], lhsT=wk_sb[:, kt, 0, :],
                                     rhs=ct[:, kt, :], start=(kt == 0),
                                     stop=(kt == KT - 1))
                    nc.tensor.matmul(kp1[:, :], lhsT=wk_sb[:, kt, 1, :],
                                     rhs=ct[:, kt, :], start=(kt == 0),
                                     stop=(kt == KT - 1))

                nc.vector.tensor_copy(kT8[:, 0, jc * 512:(jc + 1) * 512],
                                      kp0[:, :])
                nc.vector.tensor_copy(kT8[:, 1, jc * 512:(jc + 1) * 512],
                                      kp1[:, :])
                pace(2)
                ct = ct_tiles.pop(jc)
                for t in range(4):
                    jt = 4 * jc + t
                    vp = proj.tile([128, 512], F32, name="vp", tag="vp")
                    for kt in range(KT):
                        nc.tensor.matmul(
                            vp[:, 0:256],
                            lhsT=ct[:, kt, t * 128:(t + 1) * 128],
                            rhs=wv_sb[:, kt, :], start=(kt == 0),
                            stop=(kt == KT - 1))
                    nc.vector.tensor_copy(vnat[:, jt, :], vp[:, 0:256])
                    pace(1)
                pace(PACE_C - 6)
                for t in range(4):
                    pend.append((0, 4 * jc + t))
                for t in range(4):
                    pend.append((1, 4 * jc + t))

        # ============ Phase D: EV + remaining QK/exp + epilogue ============
        with tc.tile_pool(name="attn", bufs=4, space="PSUM") as attn:
            # Allocation order matters: the op tiles below cycle through the
            # same 4-buffer ring, and the ic1 tiles are normalized (freed)
            # first, so allocate ic1 into the first ring slots.
            y = {}
            for ic in (1, 0):
                for h in range(HPC):
                    y[(h, ic)] = attn.tile([128, 512], F32,
                                           name=f"y{h}_{ic}", tag="y")

            def finalize_head(h):
                # dn final: GPSIMD cross-partition all-reduce (result
                # broadcast to all partitions), then DVE reciprocal in
                # place.  No PE or Act involvement.
                nc.gpsimd.partition_all_reduce(
                    rs_sb[:, h, :], acc[:, h, :], 128,
                    bass_isa.ReduceOp.add)
                nc.vector.reciprocal(rs_sb[:, h, :], rs_sb[:, h, :])
                head_fin[h] = True

            def emit_outproj_tile(it, split_dma=False):
                # split_dma: per-half DMAs -- used for the first tile (gets
                # the output DMA stream flowing ~1us earlier) and the last
                # tile (small final transfer => end-of-kernel sem fires
                # sooner).
                ot = ostream.tile([128, D], FP16, name="ot",
                                  tag="ot", bufs=5)
                for oc in range(IC):
                    op = attn.tile([128, 512], F32, name="op", tag="y")
                    for h in range(HPC):
                        nc.tensor.matmul(
                            op[:, :],
                            lhsT=yn[:, h, it * 128:(it + 1) * 128],
                            rhs=wout_sb[:, h, oc * 512:(oc + 1) * 512],
                            start=(h == 0), stop=(h == HPC - 1))
                    # alternate Act / DVE for evictions so the two
                    # engines drain the epilogue in parallel
                    dst = ot[:, oc * 512:(oc + 1) * 512]
                    if oc == 0:
                        nc.scalar.activation(dst, op[:, :], COPY)
                    else:
                        nc.vector.tensor_copy(dst, op[:, :])
                    if split_dma:
                        # oc0's half-DMA issues from the Act queue (same
                        # engine as its eviction: no cross-engine sem hop,
                        # no SP.SEQ stacking); oc1 goes via SP
                        eng = nc.scalar if oc == 0 else nc.sync
                        eng.dma_start(
                            out=outp[it * 128:(it + 1) * 128,
                                     oc * 512:(oc + 1) * 512],
                            in_=dst)
                if not split_dma:
                    nc.sync.dma_start(
                        out=outp[it * 128:(it + 1) * 128, :],
                        in_=ot[:, :])

            # rounds 0..DRAIN-1: all four (h, ic) chains advance together.
            # By DRAIN all et tiles exist (pend drains around jt 22), so the
            # tail drains one (h, ic) chain at a time; each chain's
            # normalization mul pipelines behind the next chain's matmuls and
            # out-proj never waits on the DVE.
            DRAIN = 28
            for jt in range(DRAIN):
                for h in range(HPC):
                    if addc[h] == JT and not head_fin[h]:
                        finalize_head(h)
                for h in range(HPC):
                    et = et_tiles.pop((h, jt))
                    # ic1 first: its PSUM banks come from the kp ring whose
                    # last readers (kT8 evicts) finish earliest, so the
                    # C->D pool-handoff wait lands off the critical path
                    for ic in (1, 0):
                        nc.tensor.matmul(
                            y[(h, ic)][:, :],
                            lhsT=vnat[:, jt, h * HD:(h + 1) * HD],
                            rhs=et[:, ic * 512:(ic + 1) * 512],
                            start=(jt == 0), stop=False)
                if pend:
                    emit_qk_exp(*pend.pop(0))
            assert not pend and head_fin[0] and head_fin[1]

            def drain_chain(h, ic, mul=True):
                for jt in range(DRAIN, JT):
                    nc.tensor.matmul(
                        y[(h, ic)][:, :],
                        lhsT=vnat[:, jt, h * HD:(h + 1) * HD],
                        rhs=et_tiles[(h, jt)][:, ic * 512:(ic + 1) * 512],
                        start=False, stop=(jt == JT - 1))
                if mul:
                    emit_mul(h, ic)

            def emit_mul(h, ic):
                nc.vector.tensor_mul(
                    yn[:, h, ic * 512:(ic + 1) * 512],
                    y[(h, ic)][:, :],
                    rs_sb[:, h, ic * 512:(ic + 1) * 512])

            drain_chain(0, 1)
            drain_chain(1, 1)
            # ic0 chains run on PE now, but their DVE muls are emitted
            # after it4's eviction so the first output DMA isn't stuck
            # behind them in the DVE FIFO.
            drain_chain(0, 0, mul=False)
            drain_chain(1, 0, mul=False)
            emit_outproj_tile(4)
            emit_mul(0, 0)
            emit_mul(1, 0)
            for it in (5, 6, 7, 0, 1, 2, 3):
                emit_outproj_tile(it)
            for h in range(HPC):
                for jt in range(DRAIN, JT):
                    et_tiles.pop((h, jt))

    nc.compile()
    return nc


def get_nc():
    if "nc" not in _NC_CACHE:
        _NC_CACHE["nc"] = _build_module()
    return _NC_CACHE["nc"]


def make_in_maps(inputs):
    f16 = np.float16
    x = np.asarray(inputs["x"], dtype=np.float32)
    context = np.asarray(inputs["context"], dtype=np.float32)
    doc = np.asarray(inputs["doc_similarities"], dtype=np.float32)
    cmask = np.asarray(inputs["context_mask"])
    Wq = np.asarray(inputs["Wq"], dtype=np.float32)
    Wkv = np.asarray(inputs["Wkv"], dtype=np.float32)
    beta = float(np.asarray(inputs["beta"]))
    Wout = np.asarray(inputs["Wout"], dtype=np.float32)

    def tile_rows(a):
        # [D, C] -> [128, KT, C]: row d = kt*128 + p
        c = a.shape[1]
        return np.ascontiguousarray(
            a.reshape(KT, 128, c).transpose(1, 0, 2)).astype(f16)

    per_batch = []
    for b in range(B):
        xPb = tile_rows(np.ascontiguousarray(x[b].T))
        ctxPb = tile_rows(np.ascontiguousarray(context[b].reshape(J, D).T))
        bias = np.repeat(doc[b], CN) * beta
        bias = np.where(cmask[b].reshape(J), bias, -1e30).astype(np.float32)
        docbb = np.ascontiguousarray(bias.reshape(JT, 128).T)  # [128, JT]
        per_batch.append((xPb, ctxPb, docbb))

    def pack_kxc(w):
        c = w.shape[1]
        return np.ascontiguousarray(
            w.reshape(KT, 128, c).transpose(1, 0, 2).reshape(128, KT * c)
        ).astype(f16)

    def pack_qk(w):
        # like pack_kxc but columns reordered half-major per kt:
        # [head(2), half(2), 64] -> [half(2), head(2), 64]
        a = w.reshape(KT, 128, HPC, 2, 64).transpose(1, 0, 3, 2, 4)
        return np.ascontiguousarray(
            a.reshape(128, KT * HPC * 128)).astype(f16)

    in_maps = []
    for c in range(NCORES):
        b = c // 4
        h0 = (c % 4) * HPC
        xPb, ctxPb, docbb = per_batch[b]
        wout_c = Wout[h0 * HD:(h0 + HPC) * HD, :]
        in_maps.append({
            "xP": xPb,
            "ctxP": ctxPb,
            "wq": pack_qk(Wq[:, h0 * HD:(h0 + HPC) * HD]),
            "wk": pack_qk(Wkv[:, h0 * HD:(h0 + HPC) * HD]),
            "wv": pack_kxc(Wkv[:, D + h0 * HD:D + (h0 + HPC) * HD]),
            "wout": np.ascontiguousarray(
                wout_c.reshape(HPC, 128, D).transpose(1, 0, 2)
                .reshape(128, HPC * D)).astype(f16),
            "docb": docbb,
        })
    return in_maps


def kernel(**inputs):
    global LAST_RESULT
    nc = get_nc()
    in_maps = make_in_maps(inputs)
    res = bass_utils.run_bass_kernel_spmd(
        nc, in_maps, core_ids=list(range(NCORES))
    )
    LAST_RESULT = res
    out = np.zeros((B, N, D), dtype=np.float32)
    for c in range(NCORES):
        out[c // 4] += np.asarray(res.results[c]["outp"],
                                  dtype=np.float32)
    out += np.asarray(inputs["bout"], dtype=np.float32)
    return out
